# revision 12
# baseline (speedup 1.0000x reference)
"""Trainium2 Bass kernel for nn_GAT_n2v_mean (3-layer edge-featured GAT + mean-pool + MLP).

Strategy (hardcoded): partition edges by dst owner across 8 cores (6250 nodes
each, dst-sorted), 127-node blocks + trash slot; segment softmax/sums become
per-tile one-hot matmuls accumulating in PSUM; the segment max is replaced by a
global per-head shift (mathematically identical softmax), needing only a tiny
AllReduce-max; self-loops are applied at node level in the block epilogue; the
per-edge src features come from one batched indirect-DMA gather per block out
of an AllGathered [xs | al_s | al_d] node-feature table.
"""

import numpy as np

_PATCHED = False


def _patch_walrus():
    """Enable per-partition vector dynamic offsets in walrus codegen
    (needed for the indirect row gathers; off by default in this path)."""
    global _PATCHED
    if _PATCHED:
        return
    import concourse.bass_utils as _bu
    _orig = _bu.run_command

    def _patched(argv, **kw):
        if any("codegen" in str(a) for a in argv):
            argv = list(argv)
            i = argv.index("-i")
            argv[i:i] = ["--dge-levels=vector_dynamic_offsets"]
        return _orig(argv, **kw)

    _bu.run_command = _patched
    _PATCHED = True


# ---------------------------------------------------------------- host config
N, E, G, D = 50000, 800000, 64, 8
NPD = N // D          # nodes per device
BLK = 127             # real node slots per block (slot 127 = trash)
NB = (NPD + BLK - 1) // BLK
R = ((NB * 128) + 127) // 128 * 128   # padded local rows
EPS = 1e-5
BNC = float(1.0 / np.sqrt(1.0 + EPS))
DIMS = [(32, 4, 64), (256, 4, 128), (512, 4, 64)]

_CACHE = {}


def _prep(inputs):
    """Host-side sharding/layout prep (numpy only). Returns (in_maps, T)."""
    src_g = np.asarray(inputs["edge_index"][0], dtype=np.int64)
    dst_g = np.asarray(inputs["edge_index"][1], dtype=np.int64)
    ef = np.asarray(inputs["edge_feature"], dtype=np.float32)
    batch = np.asarray(inputs["batch"], dtype=np.int64)
    x = np.asarray(inputs["x"], dtype=np.float32)

    per_dev = []
    Tmax = 1
    for d in range(D):
        m = (dst_g // NPD) == d
        s, t, f = src_g[m], dst_g[m], ef[m]
        loc = t - d * NPD
        b = loc // BLK
        rel = loc % BLK
        order = np.argsort(b, kind="stable")
        s, f, b, rel = s[order], f[order], b[order], rel[order]
        cnt = np.bincount(b, minlength=NB)
        Tmax = max(Tmax, int(np.ceil(cnt.max() / 128)))
        per_dev.append((s, f, b, rel, cnt))
    T = Tmax

    def grow(n):      # global padded row id
        return (n // NPD) * R + (n % NPD)

    in_maps = []
    for d in range(D):
        s, f, b, rel, cnt = per_dev[d]
        idx_s = np.zeros((NB, T * 128), np.int32)
        idx_d = np.zeros((NB, T * 128), np.int32)
        relm = np.full((NB, T * 128), 127.0, np.float32)
        eaT = np.zeros((6, NB * T * 128), np.float32)
        eaR = np.zeros((NB, T * 128, 8), np.float32)
        off = np.concatenate([[0], np.cumsum(cnt)])
        for blk in range(NB):
            e0, e1 = off[blk], off[blk + 1]
            k = e1 - e0
            idx_s[blk, :k] = grow(s[e0:e1]).astype(np.int32)
            idx_d[blk, :k] = (R * d + blk * BLK + rel[e0:e1]).astype(np.int32)
            relm[blk, :k] = rel[e0:e1].astype(np.float32)
            eaT[:, blk * T * 128: blk * T * 128 + k] = f[e0:e1].T
            eaR[blk, :k, :6] = f[e0:e1]
            eaR[blk, :k, 6] = 1.0
        # per-tile transposed [NB, 128, T] layouts
        tp = lambda a: a.reshape(NB, T, 128).transpose(0, 2, 1)
        bb = np.full((NB, 128, 1), -1.0, np.float32)
        for blk in range(NB):
            lo = blk * BLK
            n = min(BLK, NPD - lo)
            if n > 0:
                bb[blk, :n, 0] = batch[d * NPD + lo: d * NPD + lo + n]
        eaRt = eaR.reshape(NB, T, 128, 8).transpose(0, 2, 1, 3) \
                  .reshape(NB, 128, T * 8)
        # record: [src T | dst T | rel T | batch 1 | eaRt 8T] as int32
        rec = np.concatenate(
            [tp(idx_s), tp(idx_d), tp(relm.view(np.int32)),
             bb.view(np.int32), eaRt.view(np.int32)], axis=2
        ).reshape(NB * 128, 11 * T + 1).copy()

        xT = np.zeros((32, R), np.float32)
        xT[:, :NPD] = x[d * NPD:(d + 1) * NPD].T

        im = {
            "xT": xT,
            "rec": rec,
            "eaT": np.ascontiguousarray(eaT),
            "iota128": np.broadcast_to(np.arange(128, dtype=np.float32),
                                       (128, 128)).copy(),
            "iota64": np.broadcast_to(np.arange(64, dtype=np.float32),
                                      (128, 64)).copy(),
            "ident": np.eye(128, dtype=np.float32),
        }
        for li, (fin, H, C) in enumerate(DIMS, 1):
            HC = H * C
            im[f"W{li}"] = np.asarray(inputs[f"W{li}"], np.float32)
            im[f"Wer{li}"] = np.broadcast_to(
                np.asarray(inputs[f"We{li}"], np.float32), (6, HC)).copy()
            im[f"aer{li}"] = np.broadcast_to(
                np.asarray(inputs[f"ae{li}"], np.float32).reshape(1, HC),
                (6, HC)).copy()
            im[f"asr{li}"] = np.broadcast_to(
                np.asarray(inputs[f"as{li}"], np.float32).reshape(1, HC),
                (128, HC)).copy()
            im[f"adr{li}"] = np.broadcast_to(
                np.asarray(inputs[f"ad{li}"], np.float32).reshape(1, HC),
                (128, HC)).copy()
            im[f"gr{li}"] = np.broadcast_to(
                np.asarray(inputs[f"g{li}"], np.float32), (128, HC)).copy()
            im[f"br{li}"] = np.broadcast_to(
                np.asarray(inputs[f"b{li}"], np.float32), (128, HC)).copy()
            im[f"bbr{li}"] = np.broadcast_to(
                np.asarray(inputs[f"bb{li}"], np.float32), (128, HC)).copy()
        im["Wf1"] = np.asarray(inputs["Wf1"], np.float32)
        im["Wf2"] = np.asarray(inputs["Wf2"], np.float32)
        im["bf1r"] = np.broadcast_to(np.asarray(inputs["bf1"], np.float32),
                                     (64, 32)).copy()
        im["gfr"] = np.broadcast_to(np.asarray(inputs["gf"], np.float32),
                                    (64, 32)).copy()
        im["bbfr"] = np.broadcast_to(np.asarray(inputs["bbf"], np.float32),
                                     (64, 32)).copy()
        im["bf2r"] = np.broadcast_to(np.asarray(inputs["bf2"], np.float32),
                                     (64, 2)).copy()
        in_maps.append(im)
    return in_maps, T


def nk2(li):
    return DIMS[li - 1][1] * DIMS[li - 1][2] // 128


# ---------------------------------------------------------------- device prog
def _build(T, dbg=False):
    import concourse.bass as bass
    import concourse.bacc as bacc
    import concourse.mybir as mybir
    import concourse.tile as tile
    from contextlib import ExitStack

    f32 = mybir.dt.float32
    i32 = mybir.dt.int32
    AO = mybir.AluOpType
    AF = mybir.ActivationFunctionType
    AX = mybir.AxisListType
    RG = [list(range(D))]
    RECW = 11 * T + 1

    nc = bacc.Bacc(None, target_bir_lowering=False, debug=True)

    # ---- I/O
    inp = {}
    def di(name, shape, dt=f32):
        inp[name] = nc.declare_dram_parameter(name, list(shape), dt,
                                              isOutput=False)
        return inp[name]

    di("xT", (32, R)); di("rec", (NB * 128, RECW), i32)
    di("eaT", (6, NB * T * 128))
    di("iota128", (128, 128)); di("iota64", (128, 64)); di("ident", (128, 128))
    for li, (fin, H, C) in enumerate(DIMS, 1):
        HC = H * C
        di(f"W{li}", (fin, HC)); di(f"Wer{li}", (6, HC)); di(f"aer{li}", (6, HC))
        di(f"asr{li}", (128, HC)); di(f"adr{li}", (128, HC))
        di(f"gr{li}", (128, HC)); di(f"br{li}", (128, HC)); di(f"bbr{li}", (128, HC))
    di("Wf1", (256, 32)); di("Wf2", (32, 2))
    di("bf1r", (64, 32)); di("gfr", (64, 32)); di("bbfr", (64, 32))
    di("bf2r", (64, 2))
    out_d = nc.declare_dram_parameter("out", [64, 2], f32, isOutput=True)
    dbg_d = {}
    if dbg:
        dbg_d["dxe1"] = nc.declare_dram_parameter("dxe1", [128, 264], f32, isOutput=True)
        dbg_d["dxe2"] = nc.declare_dram_parameter("dxe2", [128, 520], f32, isOutput=True)
        dbg_d["dem"] = nc.declare_dram_parameter("dem", [6, 128], f32, isOutput=True)
        dbg_d["dale"] = nc.declare_dram_parameter("dale", [128, 216], f32, isOutput=True)
        dbg_d["dh1"] = nc.declare_dram_parameter("dh1", [128, 256], f32, isOutput=True)
        dbg_d["dpool"] = nc.declare_dram_parameter("dpool", [64, 257], f32, isOutput=True)
        dbg_d["dmx"] = nc.declare_dram_parameter("dmx", [1, 8], f32, isOutput=True)

    # ---- internal DRAM
    emeanT_d = nc.dram_tensor("emeanT", [6, R], f32)
    ale_d = nc.dram_tensor("ale", [NB * 128, 12 * T], f32)
    Ws = [DIMS[i][1] * DIMS[i][2] + 8 for i in range(3)]
    xe_d = [nc.dram_tensor(f"xe{l}", [R, Ws[l - 1]], f32) for l in (1, 2, 3)]
    xf_d = [nc.dram_tensor(f"xf{l}", [D * R, Ws[l - 1]], f32,
                           addr_space="Shared") for l in (1, 2, 3)]
    # folded channel-major feature tables: [128, nkc, R]
    hT_d = [None,
            nc.dram_tensor("hT1", [128, 2, R], f32),
            nc.dram_tensor("hT2", [128, 4, R], f32)]
    mxi_d = [nc.dram_tensor(f"mxi{l}", [1, 8], f32) for l in (1, 2, 3)]
    mxo_d = [nc.dram_tensor(f"mxo{l}", [1, 8], f32, addr_space="Shared")
             for l in (1, 2, 3)]
    pool_i = nc.dram_tensor("pool_i", [64, 257], f32)
    pool_o = nc.dram_tensor("pool_o", [64, 257], f32, addr_space="Shared")

    with ExitStack() as ctx:
        tc = ctx.enter_context(tile.TileContext(nc))
        consts = ctx.enter_context(tc.tile_pool(name="consts", bufs=1))
        lay = ctx.enter_context(tc.tile_pool(name="lay", bufs=1))
        sb = ctx.enter_context(tc.tile_pool(name="sb", bufs=2))
        sb2 = ctx.enter_context(tc.tile_pool(name="sb2", bufs=2))
        sbg = ctx.enter_context(tc.tile_pool(name="sbg", bufs=2))
        psb = ctx.enter_context(tc.tile_pool(name="psb", bufs=2, space="PSUM"))
        pss = ctx.enter_context(tc.tile_pool(name="pss", bufs=2, space="PSUM"))
        pst = ctx.enter_context(tc.tile_pool(name="pst", bufs=3, space="PSUM"))

        io128 = consts.tile([128, 128], f32)
        nc.sync.dma_start(out=io128[:], in_=inp["iota128"][:])
        io64 = consts.tile([128, 64], f32)
        nc.sync.dma_start(out=io64[:], in_=inp["iota64"][:])
        ident = consts.tile([128, 128], f32)
        nc.sync.dma_start(out=ident[:], in_=inp["ident"][:])
        onescol = consts.tile([128, 1], f32)
        nc.any.memset(onescol[:], 1.0)
        onesrow = consts.tile([1, 128], f32)
        nc.any.memset(onesrow[:], 1.0)
        ntail = R - NB * BLK
        ztail = consts.tile([128, 4 * ntail], f32)
        nc.any.memset(ztail[:], 0.0)

        # zero the never-written pad tails (cols NB*BLK .. R)
        for l, nkc_ in ((1, 2), (2, 4)):
            nc.sync.dma_start(
                out=hT_d[l][:, :, NB * BLK:R],
                in_=ztail[:, 0:nkc_ * ntail].rearrange("p (k b) -> p k b",
                                                       k=nkc_))
        nc.sync.dma_start(out=emeanT_d[0:6, NB * BLK:R], in_=ztail[0:6, 0:ntail])

        # Ae per layer: [6, H] from Wer*aer reduced over C
        Aecat = consts.tile([6, 12], f32)
        for li, (fin, H, C) in enumerate(DIMS, 1):
            HC = H * C
            wer = sb.tile([6, HC], f32, tag="wer")
            nc.sync.dma_start(out=wer[:], in_=inp[f"Wer{li}"][:])
            aer = sb.tile([6, HC], f32, tag="aer")
            nc.sync.dma_start(out=aer[:], in_=inp[f"aer{li}"][:])
            nc.vector.tensor_tensor(out=wer[:], in0=wer[:], in1=aer[:],
                                    op=AO.mult)
            nc.vector.tensor_reduce(
                out=Aecat[:, (li - 1) * 4: li * 4],
                in_=wer[:].rearrange("p (h c) -> p h c", h=H),
                axis=AX.X, op=AO.add)

        # ---------------- precompute: emean + al_e for all layers ----------
        with tc.For_i(0, NB, 1) as i:
            st128 = i * 128
            ir = sb.tile([128, RECW], i32, tag="ir")
            nc.sync.dma_start(out=ir[:], in_=inp["rec"][bass.ds(st128, 128), :])
            eatt = sb2.tile([6, T * 128], f32, tag="eatt")
            nc.scalar.dma_start(out=eatt[:],
                                in_=inp["eaT"][:, bass.ts(i, T * 128)])
            eart = ir[:, 3 * T + 1:11 * T + 1].bitcast(f32)
            sall = sbg.tile([128, T * 128], f32, tag="sall")
            rel = ir[:, 2 * T:3 * T].bitcast(f32)
            nc.vector.tensor_tensor(
                out=sall[:].rearrange("p (t n) -> p t n", t=T),
                in0=rel.unsqueeze(2).to_broadcast([128, T, 128]),
                in1=io128[:].unsqueeze(1).to_broadcast([128, T, 128]),
                op=AO.is_equal)
            acc = pss.tile([128, 8], f32, tag="sm")
            ale3 = sb.tile([128, 12 * T], f32, tag="ale3")
            for t in range(T):
                nc.tensor.matmul(acc[:], sall[:, t * 128:(t + 1) * 128],
                                 eart[:, t * 8:(t + 1) * 8],
                                 start=(t == 0), stop=(t == T - 1))
                alp = pst.tile([128, 12], f32, tag="ps3")
                nc.tensor.matmul(alp[:], eatt[:, t * 128:(t + 1) * 128],
                                 Aecat[:], start=True, stop=True)
                for l in range(3):
                    nc.vector.tensor_copy(
                        out=ale3[:, (l * T + t) * 4:(l * T + t) * 4 + 4],
                        in_=alp[:, l * 4:l * 4 + 4])
            nc.scalar.dma_start(out=ale_d[bass.ds(st128, 128), :], in_=ale3[:])
            degc = sb.tile([128, 1], f32, tag="degc")
            nc.vector.tensor_scalar(out=degc[:], in0=acc[:, 6:7], scalar1=1.0,
                                    scalar2=None, op0=AO.max)
            nc.vector.reciprocal(out=degc[:], in_=degc[:])
            em = sb.tile([128, 8], f32, tag="em")
            nc.vector.tensor_scalar(out=em[:, 0:6], in0=acc[:, 0:6],
                                    scalar1=degc[:], scalar2=None, op0=AO.mult)
            emt_ps = pst.tile([6, 128], f32, tag="ps3")
            nc.tensor.transpose(emt_ps[:], em[:, 0:6], ident[:])
            emt = sb.tile([6, 128], f32, tag="emts")
            nc.vector.tensor_copy(out=emt[:], in_=emt_ps[:])
            nc.gpsimd.dma_start(out=emeanT_d[:, bass.ds(i * BLK, BLK)],
                                in_=emt[:, 0:BLK])

        # ---------------- layers ----------------
        pool_sb = consts.tile([64, 257], f32)
        nc.any.memset(pool_sb[:], 0.0)

        for li, (fin, H, C) in enumerate(DIMS, 1):
            HC = H * C
            W = HC + 8
            nkc = max(1, fin // 128)
            KC = fin // nkc
            xe = xe_d[li - 1]
            xf = xf_d[li - 1]

            # layer consts
            wsb = lay.tile([KC, nkc * HC], f32, tag="wsb")
            for kc in range(nkc):
                nc.sync.dma_start(out=wsb[:, kc * HC:(kc + 1) * HC],
                                  in_=inp[f"W{li}"][kc * KC:(kc + 1) * KC, :])
            asr = lay.tile([128, HC], f32, tag="asr")
            nc.sync.dma_start(out=asr[:], in_=inp[f"asr{li}"][:])
            adr = lay.tile([128, HC], f32, tag="adr")
            nc.sync.dma_start(out=adr[:], in_=inp[f"adr{li}"][:])
            ghat = lay.tile([128, HC], f32, tag="ghat")
            nc.sync.dma_start(out=ghat[:], in_=inp[f"gr{li}"][:])
            nc.vector.tensor_scalar(out=ghat[:], in0=ghat[:], scalar1=BNC,
                                    scalar2=None, op0=AO.mult)
            b2 = lay.tile([128, HC], f32, tag="b2")
            nc.sync.dma_start(out=b2[:], in_=inp[f"br{li}"][:])
            nc.vector.tensor_tensor(out=b2[:], in0=b2[:], in1=ghat[:],
                                    op=AO.mult)
            bbr = sb.tile([128, HC], f32, tag="bbr")
            nc.sync.dma_start(out=bbr[:], in_=inp[f"bbr{li}"][:])
            nc.vector.tensor_tensor(out=b2[:], in0=b2[:], in1=bbr[:], op=AO.add)

            mxrun = lay.tile([128, 8], f32, tag="mxrun")
            nc.any.memset(mxrun[:], -3e38)

            # ---- phase A (fully static): xs_ext rows ----
            for rt in range(R // 128):
                pxs = psb.tile([128, HC], f32, tag="big")
                for kc in range(nkc):
                    ht = sb.tile([KC, 128], f32, tag="ht", bufs=3)
                    if li == 1:
                        nc.sync.dma_start(
                            out=ht[:],
                            in_=inp["xT"][:, rt * 128:(rt + 1) * 128])
                    else:
                        nc.sync.dma_start(
                            out=ht[:],
                            in_=hT_d[li - 1][:, kc, rt * 128:(rt + 1) * 128])
                    nc.tensor.matmul(pxs[:], ht[:],
                                     wsb[:, kc * HC:(kc + 1) * HC],
                                     start=(kc == 0), stop=(kc == nkc - 1))
                xs = sb.tile([128, W], f32, tag="xs", bufs=3)
                nc.vector.tensor_copy(out=xs[:, 0:HC], in_=pxs[:])
                tmp = sb.tile([128, HC], f32, tag="tmpA")
                nc.vector.tensor_tensor(out=tmp[:], in0=xs[:, 0:HC], in1=asr[:],
                                        op=AO.mult)
                nc.vector.tensor_reduce(
                    out=xs[:, HC:HC + 4],
                    in_=tmp[:].rearrange("p (h c) -> p h c", h=H),
                    axis=AX.X, op=AO.add)
                nc.vector.tensor_tensor(out=tmp[:], in0=xs[:, 0:HC], in1=adr[:],
                                        op=AO.mult)
                nc.vector.tensor_reduce(
                    out=xs[:, HC + 4:HC + 8],
                    in_=tmp[:].rearrange("p (h c) -> p h c", h=H),
                    axis=AX.X, op=AO.add)
                nc.vector.tensor_tensor(out=mxrun[:], in0=mxrun[:],
                                        in1=xs[:, HC:HC + 8], op=AO.max)
                nc.sync.dma_start(out=xe[rt * 128:(rt + 1) * 128, :], in_=xs[:])

            # ---- m-hat ----
            mx_ps = pst.tile([8, 128], f32, tag="ps3")
            nc.tensor.transpose(mx_ps[:], mxrun[:], ident[:])
            mx_sb = sb.tile([8, 128], f32, tag="mxsb")
            nc.vector.tensor_copy(out=mx_sb[:], in_=mx_ps[:])
            t32 = sb.tile([32, 32], f32, tag="t32")
            nc.any.memset(t32[:], -3e38)
            nc.vector.tensor_reduce(out=t32[0:8, 0:1], in_=mx_sb[:],
                                    axis=AX.X, op=AO.max)
            v32 = sb.tile([32, 32], f32, tag="v32")
            nc.vector.transpose(out=v32[:], in_=t32[:])
            nc.sync.dma_start(out=mxi_d[li - 1][:, :], in_=v32[0:1, 0:8])
            nc.gpsimd.collective_compute(
                "AllReduce", AO.max, replica_groups=RG,
                ins=[mxi_d[li - 1][:]], outs=[mxo_d[li - 1][:]])
            mx2 = sb.tile([1, 8], f32, tag="mx2")
            nc.sync.dma_start(out=mx2[:], in_=mxo_d[li - 1][:])
            mh1 = sb.tile([1, 4], f32, tag="mh1")
            nc.vector.tensor_tensor(out=mh1[:], in0=mx2[:, 0:4],
                                    in1=mx2[:, 4:8], op=AO.add)
            mhat = lay.tile([128, 4], f32, tag="mhat")
            mh_ps = pst.tile([128, 4], f32, tag="ps3")
            nc.tensor.matmul(mh_ps[:], onesrow[:], mh1[:], start=True, stop=True)
            nc.vector.tensor_copy(out=mhat[:], in_=mh_ps[:])

            # ---- AllGather xs_ext ----
            nc.gpsimd.collective_compute(
                "AllGather", AO.bypass, replica_groups=RG,
                ins=[xe[:]], outs=[xf[:]])

            # ---- attention + aggregation + epilogue ----
            with tc.For_i(0, NB, 1) as i:
                st128 = i * 128
                stblk = i * BLK
                ir = sb.tile([128, RECW], i32, tag="ir2")
                nc.sync.dma_start(out=ir[:],
                                  in_=inp["rec"][bass.ds(st128, 128), :])
                xsl = sb.tile([128, W], f32, tag="xsl")
                nc.gpsimd.dma_start(out=xsl[:],
                                    in_=xe[bass.ds(stblk, 128), :])
                emt = sb.tile([6, 128], f32, tag="emt2")
                nc.gpsimd.dma_start(out=emt[:],
                                    in_=emeanT_d[:, bass.ds(stblk, 128)])
                alet = sb.tile([128, 4 * T], f32, tag="alet")
                nc.scalar.dma_start(
                    out=alet[:],
                    in_=ale_d[bass.ds(st128, 128),
                              (li - 1) * 4 * T: li * 4 * T])
                gat = sbg.tile([128, T * W], f32, tag="gat")
                for t in range(T):
                    nc.gpsimd.indirect_dma_start(
                        out=gat[:, t * W:(t + 1) * W], out_offset=None,
                        in_=xf[:],
                        in_offset=bass.IndirectOffsetOnAxis(ap=ir[:, t:t + 1],
                                                            axis=0))
                sall = sbg.tile([128, T * 128], f32, tag="sall")
                rel = ir[:, 2 * T:3 * T].bitcast(f32)
                nc.vector.tensor_tensor(
                    out=sall[:].rearrange("p (t n) -> p t n", t=T),
                    in0=rel.unsqueeze(2).to_broadcast([128, T, 128]),
                    in1=io128[:].unsqueeze(1).to_broadcast([128, T, 128]),
                    op=AO.is_equal)
                # batched attention logits (al_d[dst] via PE one-hot expand)
                wall = sb.tile([128, T * 4], f32, tag="wall")
                nc.vector.tensor_tensor(
                    out=wall[:],
                    in0=gat[:].rearrange("p (t w) -> p t w", t=T)[:, :, HC:HC + 4],
                    in1=alet[:], op=AO.add)
                for t in range(T):
                    snp = pst.tile([128, 128], f32, tag="ps3")
                    nc.tensor.transpose(snp[:], sall[:, t * 128:(t + 1) * 128],
                                        ident[:])
                    sns = sb.tile([128, 128], f32, tag="sns", bufs=3)
                    nc.vector.tensor_copy(out=sns[:], in_=snp[:])
                    atp = pss.tile([128, 4], f32, tag="sm")
                    nc.tensor.matmul(atp[:], sns[:], xsl[:, HC + 4:HC + 8],
                                     start=True, stop=True)
                    nc.vector.tensor_tensor(out=wall[:, t * 4:(t + 1) * 4],
                                            in0=wall[:, t * 4:(t + 1) * 4],
                                            in1=atp[:], op=AO.add)
                lk = sb.tile([128, T * 4], f32, tag="lk")
                nc.vector.tensor_scalar(out=lk[:], in0=wall[:], scalar1=0.2,
                                        scalar2=None, op0=AO.mult)
                nc.vector.tensor_tensor(out=wall[:], in0=wall[:], in1=lk[:],
                                        op=AO.max)
                nc.vector.tensor_tensor(
                    out=wall[:].rearrange("p (t h) -> p t h", t=T),
                    in0=wall[:].rearrange("p (t h) -> p t h", t=T),
                    in1=mhat[:].unsqueeze(1).to_broadcast([128, T, 4]),
                    op=AO.subtract)
                nc.scalar.activation(out=wall[:], in_=wall[:], func=AF.Exp)
                # aggregation
                nps = psb.tile([128, HC], f32, tag="big")
                dps = pss.tile([128, 4], f32, tag="sm")
                for t in range(T):
                    val = sb.tile([128, HC], f32, tag="val", bufs=3)
                    gslc = gat[:, t * W:t * W + HC]
                    for h in range(H):
                        if h % 2 == 0:
                            nc.vector.tensor_scalar(
                                out=val[:, h * C:(h + 1) * C],
                                in0=gslc[:, h * C:(h + 1) * C],
                                scalar1=wall[:, t * 4 + h:t * 4 + h + 1],
                                scalar2=None, op0=AO.mult)
                        else:
                            nc.scalar.activation(
                                out=val[:, h * C:(h + 1) * C],
                                in_=gslc[:, h * C:(h + 1) * C],
                                func=AF.Copy,
                                scale=wall[:, t * 4 + h:t * 4 + h + 1])
                    nc.tensor.matmul(nps[:], sall[:, t * 128:(t + 1) * 128],
                                     val[:], start=(t == 0), stop=(t == T - 1))
                    nc.tensor.matmul(dps[:], sall[:, t * 128:(t + 1) * 128],
                                     wall[:, t * 4:(t + 1) * 4],
                                     start=(t == 0), stop=(t == T - 1))
                # epilogue: self loop + normalize + BN + ELU
                aesp = pst.tile([128, 4], f32, tag="ps3")
                nc.tensor.matmul(aesp[:], emt[:], Aecat[:, (li - 1) * 4:li * 4],
                                 start=True, stop=True)
                als = sb.tile([128, 4], f32, tag="als")
                nc.vector.tensor_tensor(out=als[:], in0=xsl[:, HC:HC + 4],
                                        in1=xsl[:, HC + 4:HC + 8], op=AO.add)
                nc.vector.tensor_tensor(out=als[:], in0=als[:], in1=aesp[:],
                                        op=AO.add)
                lk2 = sb.tile([128, 4], f32, tag="lk2")
                nc.vector.tensor_scalar(out=lk2[:], in0=als[:], scalar1=0.2,
                                        scalar2=None, op0=AO.mult)
                nc.vector.tensor_tensor(out=als[:], in0=als[:], in1=lk2[:],
                                        op=AO.max)
                nc.vector.tensor_tensor(out=als[:], in0=als[:], in1=mhat[:],
                                        op=AO.subtract)
                nc.scalar.activation(out=als[:], in_=als[:], func=AF.Exp)
                den = sb.tile([128, 4], f32, tag="den")
                nc.vector.tensor_tensor(out=den[:], in0=dps[:], in1=als[:],
                                        op=AO.add)
                nc.vector.reciprocal(out=den[:], in_=den[:])
                hh = sb2.tile([128, HC], f32, tag="hh")
                for h in range(H):
                    hsl = hh[:, h * C:(h + 1) * C]
                    nc.vector.scalar_tensor_tensor(
                        out=hsl, in0=xsl[:, h * C:(h + 1) * C],
                        scalar=als[:, h:h + 1],
                        in1=nps[:, h * C:(h + 1) * C],
                        op0=AO.mult, op1=AO.add)
                    nc.vector.tensor_scalar(
                        out=hsl, in0=hsl, scalar1=den[:, h:h + 1],
                        scalar2=None, op0=AO.mult)
                nc.vector.tensor_tensor(out=hh[:], in0=hh[:], in1=ghat[:],
                                        op=AO.mult)
                nc.vector.tensor_tensor(out=hh[:], in0=hh[:], in1=b2[:],
                                        op=AO.add)
                zn = sb2.tile([128, HC], f32, tag="zn")
                nc.vector.tensor_scalar(out=zn[:], in0=hh[:], scalar1=0.0,
                                        scalar2=None, op0=AO.min)
                nc.scalar.activation(out=zn[:], in_=zn[:], func=AF.Exp)
                rl = sb2.tile([128, HC], f32, tag="rl")
                nc.scalar.activation(out=rl[:], in_=hh[:], func=AF.Relu)
                nc.vector.scalar_tensor_tensor(
                    out=hh[:], in0=zn[:], scalar=-1.0, in1=rl[:],
                    op0=AO.add, op1=AO.add)
                if li < 3:
                    hta = sb.tile([128, nk2(li) * 128], f32, tag="hta")
                    for ch in range(nk2(li)):
                        trp = pst.tile([128, 128], f32, tag="ps3")
                        nc.tensor.transpose(
                            trp[:], hh[:, ch * 128:(ch + 1) * 128], ident[:])
                        nc.vector.tensor_copy(
                            out=hta[:, ch * 128:(ch + 1) * 128], in_=trp[:])
                    nc.scalar.dma_start(
                        out=hT_d[li][:, :, bass.ds(stblk, BLK)],
                        in_=hta[:].rearrange("p (k b) -> p k b",
                                             k=nk2(li))[:, :, 0:BLK])
                else:
                    bcol = ir[:, 3 * T:3 * T + 1].bitcast(f32)
                    bt = sb.tile([128, 64], f32, tag="bt")
                    nc.vector.tensor_tensor(out=bt[:],
                                            in0=bcol.to_broadcast([128, 64]),
                                            in1=io64[:], op=AO.is_equal)
                    pps = pst.tile([64, 257], f32, tag="ps3")
                    nc.tensor.matmul(pps[:, 0:HC], bt[:], hh[:],
                                     start=True, stop=True)
                    nc.tensor.matmul(pps[:, HC:HC + 1], bt[:], onescol[:],
                                     start=True, stop=True)
                    nc.vector.tensor_tensor(out=pool_sb[:], in0=pool_sb[:],
                                            in1=pps[:], op=AO.add)

        if dbg:
            nc.sync.dma_start(out=dbg_d["dxe1"][:, :], in_=xe_d[0][0:128, :])
            nc.sync.dma_start(out=dbg_d["dxe2"][:, :], in_=xe_d[1][0:128, :])
            nc.sync.dma_start(out=dbg_d["dem"][:, :], in_=emeanT_d[:, 0:128])
            nc.sync.dma_start(out=dbg_d["dale"][:, :], in_=ale_d[0:128, :])
            dh1t = sb.tile([128, 256], f32, tag="dh1t")
            nc.sync.dma_start(out=dh1t[:, 0:128], in_=hT_d[1][:, 0, 0:128])
            nc.sync.dma_start(out=dh1t[:, 128:256], in_=hT_d[1][:, 1, 0:128])
            nc.sync.dma_start(out=dbg_d["dh1"][:, :], in_=dh1t[:])
            nc.sync.dma_start(out=dbg_d["dpool"][:, :], in_=pool_sb[:])
            nc.sync.dma_start(out=dbg_d["dmx"][:, :], in_=mxo_d[0][:, :])

        # ---------------- final MLP ----------------
        nc.sync.dma_start(out=pool_i[:], in_=pool_sb[:])
        nc.gpsimd.collective_compute("AllReduce", AO.add, replica_groups=RG,
                                     ins=[pool_i[:]], outs=[pool_o[:]])
        pool2 = sb.tile([64, 257], f32, tag="pool2")
        nc.sync.dma_start(out=pool2[:], in_=pool_o[:])
        cnt = sb.tile([64, 1], f32, tag="cnt")
        nc.vector.tensor_scalar(out=cnt[:], in0=pool2[:, 256:257], scalar1=1.0,
                                scalar2=None, op0=AO.max)
        nc.vector.reciprocal(out=cnt[:], in_=cnt[:])
        nc.vector.tensor_scalar(out=pool2[:, 0:256], in0=pool2[:, 0:256],
                                scalar1=cnt[:], scalar2=None, op0=AO.mult)
        pts = sb.tile([128, 128], f32, tag="pts")
        for ch in range(2):
            ptp = pst.tile([128, 64], f32, tag="ps3")
            nc.tensor.transpose(ptp[:], pool2[:, ch * 128:(ch + 1) * 128],
                                ident[0:64, 0:64])
            nc.vector.tensor_copy(out=pts[:, ch * 64:(ch + 1) * 64],
                                  in_=ptp[:])
        wf1 = sb.tile([128, 64], f32, tag="wf1")
        for ch in range(2):
            nc.sync.dma_start(out=wf1[:, ch * 32:(ch + 1) * 32],
                              in_=inp["Wf1"][ch * 128:(ch + 1) * 128, :])
        z1p = pst.tile([64, 32], f32, tag="ps3")
        for ch in range(2):
            nc.tensor.matmul(z1p[:], pts[:, ch * 64:(ch + 1) * 64],
                             wf1[:, ch * 32:(ch + 1) * 32],
                             start=(ch == 0), stop=(ch == 1))
        gf = sb.tile([64, 32], f32, tag="gf")
        nc.sync.dma_start(out=gf[:], in_=inp["gfr"][:])
        nc.vector.tensor_scalar(out=gf[:], in0=gf[:], scalar1=BNC,
                                scalar2=None, op0=AO.mult)
        b2f = sb.tile([64, 32], f32, tag="b2f")
        nc.sync.dma_start(out=b2f[:], in_=inp["bf1r"][:])
        nc.vector.tensor_tensor(out=b2f[:], in0=b2f[:], in1=gf[:], op=AO.mult)
        bbf = sb.tile([64, 32], f32, tag="bbf")
        nc.sync.dma_start(out=bbf[:], in_=inp["bbfr"][:])
        nc.vector.tensor_tensor(out=b2f[:], in0=b2f[:], in1=bbf[:], op=AO.add)
        zf = sb.tile([64, 32], f32, tag="zf")
        nc.vector.tensor_tensor(out=zf[:], in0=z1p[:], in1=gf[:], op=AO.mult)
        nc.vector.tensor_tensor(out=zf[:], in0=zf[:], in1=b2f[:], op=AO.add)
        zn2 = sb.tile([64, 32], f32, tag="zn2")
        nc.vector.tensor_scalar(out=zn2[:], in0=zf[:], scalar1=0.0,
                                scalar2=None, op0=AO.min)
        nc.scalar.activation(out=zn2[:], in_=zn2[:], func=AF.Exp)
        rl2 = sb.tile([64, 32], f32, tag="rl2")
        nc.scalar.activation(out=rl2[:], in_=zf[:], func=AF.Relu)
        nc.vector.scalar_tensor_tensor(out=zf[:], in0=zn2[:], scalar=-1.0,
                                       in1=rl2[:], op0=AO.add, op1=AO.add)
        ztp = pst.tile([32, 64], f32, tag="ps3")
        nc.tensor.transpose(ztp[:], zf[:], ident[0:64, 0:64])
        zts = sb.tile([32, 64], f32, tag="zts")
        nc.vector.tensor_copy(out=zts[:], in_=ztp[:])
        wf2 = sb.tile([32, 2], f32, tag="wf2")
        nc.sync.dma_start(out=wf2[:], in_=inp["Wf2"][:])
        z2p = pst.tile([64, 2], f32, tag="ps3")
        nc.tensor.matmul(z2p[:], zts[:], wf2[:], start=True, stop=True)
        bf2 = sb.tile([64, 2], f32, tag="bf2")
        nc.sync.dma_start(out=bf2[:], in_=inp["bf2r"][:])
        z2 = sb.tile([64, 2], f32, tag="z2")
        nc.vector.tensor_tensor(out=z2[:], in0=z2p[:], in1=bf2[:], op=AO.add)
        mrow = sb.tile([64, 1], f32, tag="mrow")
        nc.vector.tensor_reduce(out=mrow[:], in_=z2[:], axis=AX.X, op=AO.max)
        nc.vector.tensor_scalar(out=z2[:], in0=z2[:], scalar1=mrow[:],
                                scalar2=None, op0=AO.subtract)
        ez = sb.tile([64, 2], f32, tag="ez")
        nc.scalar.activation(out=ez[:], in_=z2[:], func=AF.Exp)
        ssum = sb.tile([64, 1], f32, tag="ssum")
        nc.vector.tensor_reduce(out=ssum[:], in_=ez[:], axis=AX.X, op=AO.add)
        nc.scalar.activation(out=ssum[:], in_=ssum[:], func=AF.Ln)
        nc.vector.tensor_scalar(out=z2[:], in0=z2[:], scalar1=ssum[:],
                                scalar2=None, op0=AO.subtract)
        nc.sync.dma_start(out=out_d[:, :], in_=z2[:])

    nc.compile()
    return nc


# ---------------------------------------------------------------- entry point
def kernel(**inputs):
    _patch_walrus()
    in_maps, T = _prep(inputs)
    if T not in _CACHE:
        _CACHE[T] = _build(T)
    nc = _CACHE[T]
    from concourse.bass_utils import run_bass_kernel_spmd
    res = run_bass_kernel_spmd(nc, in_maps, list(range(D))).results
    return np.asarray(res[0]["out"], dtype=np.float32)
